# revision 7
# baseline (speedup 1.0000x reference)
"""GraphSAGE (3-layer, mean-agg) on 8 Trainium2 NeuronCores — v2.

Strategy (nodes sharded by id range, weights replicated, edges
partitioned by destination owner):
  - Node space PERMUTED into (core, group, slot) blocks: each core's
    6250 destinations greedily packed into G groups of <=128 dsts whose
    edges fit 1024 slots per src-half (halves = node-id halves). Each
    group owns a static 128-row block of the layout.
  - The halo replica (h of all nodes) lives in HBM in **fp8e4**: layer
    inputs are gathered per edge slot with 256B descriptors (half the
    SWDGE drain cost of bf16), and the per-layer AllGather moves half
    the bytes. The root path (h of own shard) stays bf16 feat-major in
    SBUF, so only the neighbor-mean term sees fp8 quantization.
  - Gathers are batched 4 groups per dma_gather call (4096 descriptors)
    to amortize the ~1us fixed SWDGE emission cost per call.
  - Segment-sum = one-hot fp8 matmuls in **DoubleRow** perf mode: both
    operands fp8, contracting 256 edge slots per instruction -> 8
    matmuls per group instead of 16.
  - Per-unit constant tables (stab one-hot, gather indices) stream as
    few large HWDGE transfers; inv_deg is resident (one [128, G] tile).
  - Dense layer y^T = Wl^T agg^T + Wr^T h^T (+b, relu) fused per group
    with stationary weights; relu lands directly in the next layer's
    feat-major root buffer; row-major fp8 copy feeds the halo shard.
"""

import sys

sys.path.insert(0, "/opt/trn_rl_repo")

import numpy as np
import ml_dtypes

import concourse.bass as bass
import concourse.bacc as bacc
import concourse.tile as tile
import concourse.mybir as mybir
from concourse.bass_utils import run_bass_kernel_spmd

BF16 = ml_dtypes.bfloat16

N = 50000
E = 800000
D = 256
L = 3
P = 8
NSH = N // P            # 6250 nodes per core
CAPB = 8                # gather blocks (of 128 slots) per src-half per group
CAP = CAPB * 128        # 1024 edge slots per src-half per group
NBLK = 2 * CAPB         # 16 segment blocks per group
U = 1                   # groups per gather call / per table-load unit


def _pack_idx16(idx):
    """Pack idx list (len multiple of 16) into [128, len/16] int16 layout:
    slot j -> [j % 16, j // 16], replicated to all 8 Q7-core stripes."""
    n = idx.shape[0]
    return np.tile(idx.reshape(n // 16, 16).T, (8, 1)).astype(np.int16)


def _preprocess(x, edge_index):
    """Group edges by dst windows; build permuted layout + gather/segment
    tables. Returns dict of host arrays + layout info."""
    src = edge_index[0].astype(np.int64)
    dst = edge_index[1].astype(np.int64)
    deg = np.bincount(dst, minlength=N).astype(np.float64)
    inv_deg = (1.0 / np.maximum(deg, 1.0)).astype(np.float32)

    cores = []
    for c in range(P):
        lo, hi = c * NSH, (c + 1) * NSH
        m = (dst >= lo) & (dst < hi)
        s_c = src[m]
        d_c = dst[m] - lo
        order = np.argsort(d_c, kind="stable")
        s_c, d_c = s_c[order], d_c[order]
        isB = s_c >= N // 2
        degA = np.bincount(d_c[~isB], minlength=NSH)
        degB = np.bincount(d_c[isB], minlength=NSH)
        assert degA.max() <= CAP and degB.max() <= CAP

        groups = []  # (base, end)
        base, ca, cb = 0, 0, 0
        for dd in range(NSH):
            da, db = degA[dd], degB[dd]
            if (ca + da > CAP) or (cb + db > CAP) or (dd - base >= 128):
                groups.append((base, dd))
                base, ca, cb = dd, 0, 0
            ca += da
            cb += db
        groups.append((base, NSH))
        cores.append((groups, s_c, d_c, isB))

    G = max(len(g[0]) for g in cores)
    G = ((G + U - 1) // U) * U      # pad to unit multiple
    GP = G * 128                    # permuted rows per core
    NP = P * GP                     # total permuted rows
    PHALF = NP // 2                 # src-half split (core-major == id split)
    assert PHALF < 32768

    # node id -> permuted row
    perm = np.full(N, -1, dtype=np.int64)
    for c in range(P):
        groups = cores[c][0]
        for g, (base, end) in enumerate(groups):
            span = end - base
            perm[c * NSH + base : c * NSH + end] = (
                c * GP + g * 128 + np.arange(span)
            )
    assert (perm >= 0).all()

    gidxA = np.zeros((P, 128, G * CAP // 16), dtype=np.int16)
    gidxB = np.zeros((P, 128, G * CAP // 16), dtype=np.int16)
    s_all = np.zeros((P, 128, G * NBLK, 128), dtype=np.float32)
    invd_all = np.ones((P, 128, G), dtype=np.float32)
    for c in range(P):
        groups, s_c, d_c, isB = cores[c]
        ps_c = perm[s_c]
        eA = np.nonzero(~isB)[0]
        eB = np.nonzero(isB)[0]
        dA = d_c[eA]
        dB = d_c[eB]
        for g in range(G):
            if g < len(groups):
                base, end = groups[g]
            else:
                base, end = 0, 0
            idxA = np.zeros(CAP, dtype=np.int16)
            idxB = np.zeros(CAP, dtype=np.int16)
            loA, hiA = np.searchsorted(dA, base), np.searchsorted(dA, end)
            loB, hiB = np.searchsorted(dB, base), np.searchsorted(dB, end)
            kA, kB = hiA - loA, hiB - loB
            assert kA <= CAP and kB <= CAP
            idxA[:kA] = ps_c[eA[loA:hiA]]
            idxB[:kB] = ps_c[eB[loB:hiB]] - PHALF
            cs = slice(g * CAP // 16, (g + 1) * CAP // 16)
            gidxA[c, :, cs] = _pack_idx16(idxA)
            gidxB[c, :, cs] = _pack_idx16(idxB)

            if g < len(groups):
                invd_all[c, : end - base, g] = inv_deg[
                    c * NSH + base : c * NSH + end
                ]
            if kA:
                jj = np.arange(kA)
                dloc = d_c[eA[loA:hiA]] - base
                s_all[c, jj % 128, g * NBLK + jj // 128, dloc] = 1.0
            if kB:
                jj = np.arange(kB)
                dloc = d_c[eB[loB:hiB]] - base
                s_all[c, jj % 128, g * NBLK + CAPB + jj // 128, dloc] = 1.0

    return {
        "G": G,
        "perm": perm,
        "gidxA": gidxA,
        "gidxB": gidxB,
        "stab": s_all.astype(mybir.dt.np(mybir.dt.float8e4)),
        "invd": invd_all,
    }


def _build_program(G):
    """Build + compile the single SPMD program (parametrized by group count)."""
    GP = G * 128
    NP = P * GP
    PHALF = NP // 2
    NU = G // U
    nc = bacc.Bacc("TRN2", target_bir_lowering=False, debug=False, num_devices=P,
                   num_swdge_queues=4)
    f32, bf16, i16 = mybir.dt.float32, mybir.dt.bfloat16, mybir.dt.int16
    fp8 = mybir.dt.float8e4
    DR = mybir.MatmulPerfMode.DoubleRow
    ADD, MAX = mybir.AluOpType.add, mybir.AluOpType.max

    xh = nc.dram_tensor("xh", [NP, D], fp8, kind="ExternalInput")
    xsT = nc.dram_tensor("xsT", [128, 2, GP], bf16, kind="ExternalInput")
    wl = nc.dram_tensor("wl", [L, 2, 128, D], bf16, kind="ExternalInput")
    wr = nc.dram_tensor("wr", [L, 2, 128, D], bf16, kind="ExternalInput")
    bias = nc.dram_tensor("bias", [L, 2, 128, 1], f32, kind="ExternalInput")
    ident = nc.dram_tensor("ident", [128, 128], bf16, kind="ExternalInput")
    gidxA = nc.dram_tensor("gidxA", [128, G * CAP // 16], i16, kind="ExternalInput")
    gidxB = nc.dram_tensor("gidxB", [128, G * CAP // 16], i16, kind="ExternalInput")
    stab = nc.dram_tensor("stab", [128, G * NBLK, 128], fp8, kind="ExternalInput")
    invd = nc.dram_tensor("invd", [128, G], f32, kind="ExternalInput")
    out = nc.dram_tensor("out", [GP, D], f32, kind="ExternalOutput")

    UCOL = U * CAP // 16        # gidx int16 columns per unit

    with tile.TileContext(nc) as tc:
        with (
            tc.tile_pool(name="dram", bufs=1, space="DRAM") as dram,
            tc.tile_pool(name="const", bufs=1) as const,
            tc.tile_pool(name="xt", bufs=2) as xtp,
            tc.tile_pool(name="ga", bufs=3) as gap,
            tc.tile_pool(name="gb", bufs=3) as gbp,
            tc.tile_pool(name="sp", bufs=2) as sp,
            tc.tile_pool(name="gi", bufs=4) as gip,
            tc.tile_pool(name="stage", bufs=6) as stage,
            tc.tile_pool(name="pa", bufs=2, space="PSUM") as pap,
            tc.tile_pool(name="py", bufs=2, space="PSUM") as pyp,
            tc.tile_pool(name="pt", bufs=2, space="PSUM") as ptp,
            tc.tile_pool(name="pt2", bufs=2, space="PSUM") as ptp2,
        ):
            hsh_d = [
                dram.tile([GP, D], fp8, tag=f"hsh{i}", name=f"hsh{i}")
                for i in range(2)
            ]
            hfl_d = [
                dram.tile([NP, D], fp8, tag=f"hfl{i}", name=f"hfl{i}",
                          addr_space="Shared")
                for i in range(2)
            ]

            # resident constants
            w_sb = {}
            for l in range(L):
                for k in range(2):
                    t = const.tile([128, D], bf16, tag=f"wl{l}{k}", name=f"wl{l}{k}")
                    nc.sync.dma_start(t[:], wl[l, k])
                    w_sb[("l", l, k)] = t
                    t = const.tile([128, D], bf16, tag=f"wr{l}{k}", name=f"wr{l}{k}")
                    nc.sync.dma_start(t[:], wr[l, k])
                    w_sb[("r", l, k)] = t
            b_sb = {}
            for l in range(L):
                for mh in range(2):
                    t = const.tile([128, 1], f32, tag=f"b{l}{mh}", name=f"b{l}{mh}")
                    nc.sync.dma_start(t[:], bias[l, mh])
                    b_sb[(l, mh)] = t
            id_sb = const.tile([128, 128], bf16, tag="ident", name="id_sb")
            nc.sync.dma_start(id_sb[:], ident[:])
            iv_sb = const.tile([128, G], f32, tag="ivall", name="iv_sb")
            nc.sync.dma_start(iv_sb[:], invd[:])

            # persistent transposed-shard buffers (root path, feat-major)
            xt = [xtp.tile([128, 2, GP], bf16, tag="xt", name=f"xt{i}")
                  for i in range(2)]
            nc.sync.dma_start(xt[0][:], xsT[:])

            cur = 0
            # SWDGE queue must track the global Pool-DMA instruction index:
            # Tile binds DMASW sem lanes round-robin (mod 8) over those
            # instructions and each lane must stay on one queue (mod 4).
            kq = 0
            for l in range(L):
                src_t = xh if l == 0 else hfl_d[l - 1]
                for u in range(NU):
                    st = sp.tile([128, U * NBLK, 128], fp8, name="st")
                    nc.sync.dma_start(
                        st[:], stab[:, u * U * NBLK : (u + 1) * U * NBLK, :]
                    )
                    giA = gip.tile([128, UCOL], i16, name="giA", tag="gi")
                    giB = gip.tile([128, UCOL], i16, name="giB", tag="gi")
                    nc.sync.dma_start(giA[:], gidxA[:, u * UCOL : (u + 1) * UCOL])
                    nc.sync.dma_start(giB[:], gidxB[:, u * UCOL : (u + 1) * UCOL])
                    ga = gap.tile([128, U * CAPB, D], fp8, name="ga")
                    gb = gbp.tile([128, U * CAPB, D], fp8, name="gb")
                    nc.gpsimd.dma_gather(
                        ga[:], src_t[0:PHALF, :], giA[:],
                        U * CAP, U * CAP, D, queue_num=kq % 4,
                    )
                    kq += 1
                    nc.gpsimd.dma_gather(
                        gb[:], src_t[PHALF:NP, :], giB[:],
                        U * CAP, U * CAP, D, queue_num=kq % 4,
                    )
                    kq += 1

                    for j in range(U):
                        gg = u * U + j
                        gs = slice(gg * 128, (gg + 1) * 128)
                        # segment-sum: agg[dst_slot, feat] in PSUM, fp8
                        # DoubleRow (256 slots per matmul)
                        pa = pap.tile([128, D], f32, name="pa")
                        for q in range(CAPB // 2):
                            nc.tensor.matmul(
                                pa[:],
                                st[:, j * NBLK + 2 * q : j * NBLK + 2 * q + 2, :],
                                ga[:, j * CAPB + 2 * q : j * CAPB + 2 * q + 2, :],
                                start=(q == 0),
                                stop=False,
                                perf_mode=DR,
                            )
                        for q in range(CAPB // 2):
                            nc.tensor.matmul(
                                pa[:],
                                st[:, j * NBLK + CAPB + 2 * q
                                     : j * NBLK + CAPB + 2 * q + 2, :],
                                gb[:, j * CAPB + 2 * q : j * CAPB + 2 * q + 2, :],
                                start=False,
                                stop=(q == CAPB // 2 - 1),
                                perf_mode=DR,
                            )
                        # mean scale (per-dst inv_deg) + downcast to bf16
                        ab = stage.tile([128, D], bf16, name="ab", tag="ab")
                        nc.vector.tensor_scalar_mul(
                            ab[:], pa[:], iv_sb[:, gg : gg + 1]
                        )
                        # transpose agg to feat-major
                        aT = stage.tile([128, 2, 128], bf16, name="aT", tag="aT")
                        for k in range(2):
                            pt = ptp.tile([128, 128], bf16, name="pt")
                            nc.tensor.transpose(
                                pt[:], ab[:, k * 128 : (k + 1) * 128], id_sb[:]
                            )
                            nc.vector.tensor_copy(aT[:, k, :], pt[:])

                        # dense: yT[mh] = sum_k Wl[k,mh]^T aggT[k] + Wr[k,mh]^T xT[k]
                        py = pyp.tile([128, 2, 128], f32, name="py")
                        for mh in range(2):
                            ms = slice(mh * 128, (mh + 1) * 128)
                            nc.tensor.matmul(py[:, mh, :], w_sb[("l", l, 0)][:, ms],
                                             aT[:, 0, :], start=True, stop=False)
                            nc.tensor.matmul(py[:, mh, :], w_sb[("l", l, 1)][:, ms],
                                             aT[:, 1, :], start=False, stop=False)
                            nc.tensor.matmul(py[:, mh, :], w_sb[("r", l, 0)][:, ms],
                                             xt[cur][:, 0, gs], start=False, stop=False)
                            nc.tensor.matmul(py[:, mh, :], w_sb[("r", l, 1)][:, ms],
                                             xt[cur][:, 1, gs], start=False, stop=True)
                        if l < L - 1:
                            # bias+relu lands straight in the next layer's
                            # feat-major root buffer
                            yT_view = [xt[1 - cur][:, mh, gs] for mh in range(2)]
                            for mh in range(2):
                                nc.vector.tensor_scalar(
                                    yT_view[mh], py[:, mh, :],
                                    b_sb[(l, mh)][:], 0.0, ADD, MAX,
                                )
                        else:
                            yT = stage.tile([128, 2, 128], bf16, name="yT", tag="yT")
                            yT_view = [yT[:, mh, :] for mh in range(2)]
                            for mh in range(2):
                                nc.vector.tensor_scalar_add(
                                    yT_view[mh], py[:, mh, :], b_sb[(l, mh)][:]
                                )
                        # back to row-major for the halo replica / output
                        ydt = fp8 if l < L - 1 else f32
                        yr = stage.tile([128, D], ydt, name="yr",
                                        tag=f"yr{l == L - 1}")
                        for mh in range(2):
                            pt2 = ptp2.tile([128, 128], bf16, name="pt2")
                            nc.tensor.transpose(pt2[:], yT_view[mh], id_sb[:])
                            nc.vector.tensor_copy(
                                yr[:, mh * 128 : (mh + 1) * 128], pt2[:]
                            )
                        if l < L - 1:
                            nc.sync.dma_start(hsh_d[l][gs, :], yr[:])
                        else:
                            nc.sync.dma_start(out[gs, :], yr[:])

                if l < L - 1:
                    # bitcast fp8 -> bf16 for the collective: AllGather is a
                    # byte mover and the collectives runtime may not know fp8
                    nc.gpsimd.collective_compute(
                        "AllGather",
                        mybir.AluOpType.bypass,
                        replica_groups=[list(range(P))],
                        ins=[hsh_d[l][:].bitcast(bf16)],
                        outs=[hfl_d[l][:].bitcast(bf16)],
                    )
                    cur = 1 - cur

    nc.compile()
    return nc


_CACHE = {}


def _get_program(G):
    if G not in _CACHE:
        _CACHE[G] = _build_program(G)
    return _CACHE[G]


LAST_EXEC_NS = None


def kernel(x, edge_index, Wl, Wr, b, _trace=False):
    global LAST_EXEC_NS
    x = np.asarray(x, dtype=np.float32)
    edge_index = np.asarray(edge_index)
    Wl = np.asarray(Wl, dtype=np.float32)
    Wr = np.asarray(Wr, dtype=np.float32)
    b = np.asarray(b, dtype=np.float32)

    pre = _preprocess(x, edge_index)
    G = pre["G"]
    GP = G * 128
    NP = P * GP
    nc = _get_program(G)

    FP8NP = mybir.dt.np(mybir.dt.float8e4)
    # permuted full replica (fp8 for the gather/halo path)
    xh32 = np.zeros((NP, D), dtype=np.float32)
    xh32[pre["perm"]] = x
    xh = xh32.astype(FP8NP)

    wl_h = np.ascontiguousarray(Wl.reshape(L, 2, 128, D).astype(BF16))
    wr_h = np.ascontiguousarray(Wr.reshape(L, 2, 128, D).astype(BF16))
    b_h = np.ascontiguousarray(b.reshape(L, 2, 128, 1).astype(np.float32))
    id_h = np.eye(128, dtype=BF16)

    in_maps = []
    for c in range(P):
        xs = xh32[c * GP : (c + 1) * GP]
        xsT = np.ascontiguousarray(
            xs.T.reshape(2, 128, GP).transpose(1, 0, 2).astype(BF16)
        )
        in_maps.append(
            {
                "xh": xh,
                "xsT": xsT,
                "wl": wl_h,
                "wr": wr_h,
                "bias": b_h,
                "ident": id_h,
                "gidxA": pre["gidxA"][c],
                "gidxB": pre["gidxB"][c],
                "stab": pre["stab"][c],
                "invd": pre["invd"][c],
            }
        )

    res = run_bass_kernel_spmd(
        nc, in_maps, core_ids=list(range(P)), trace=bool(_trace)
    )
    LAST_EXEC_NS = res.exec_time_ns

    out_full = np.empty((N, D), dtype=np.float32)
    outs = np.concatenate([res.results[c]["out"] for c in range(P)], axis=0)
    out_full[:] = outs[pre["perm"]]
    return out_full
